# revision 60
# baseline (speedup 1.0000x reference)
"""AVWGCN (adaptive vertex-wise GCN) Bass/Tile kernel for 8 Trainium2 NeuronCores.

Sharding: data-parallel over batch B=64 -> 8 cores x 8 batches. Every core
computes the full adaptive adjacency (cheap) and its batch slice of the
graph conv. No collectives.

Math (per core, x is the [8, 2048, 64] batch slice):
  T    = exp(relu(E @ E^T))            # SBUF-resident bf16, never touches DRAM
  r    = 1 / rowsum(T)                 # free accum on the exp pass
  z1   = r * (T @ x)                   # supports @ x            (cheb k=1)
  z2   = 2r * (T @ z1)                 # 2*supports@z1           (cheb k=2; -I folded into W0)
  y[bn, (d,o)] = xT@(W0-W2) + z1T@W1 + z2T@W2
  out[b,n,o] = sum_d E[n,d] * y[bn,(d,o)] + E[n,:] @ bias_pool

Phase layout (keeps the PE stream dependency-free and warm):
  S1: x load/cast + adjacency build (relu on DVE, exp+rowsum on ACT) with
      the z1 q0-quarter accumulating one tile behind the exp pipeline.
  S2: z1 q1-3 sweeps (pure PE); z2 sweeps with the x-transposes (XT)
      interleaved; evictions + stacked bf16 copies on ACT.
  S4: per tile: 8 PE transposes -> stA; pure-GEMM units (2 batches x 8 d)
      into 2-bank PSUM tiles, bulk bf16 evictions to an SBUF ring on
      ACT/DVE; then the whole d-contraction + bias as 17 diag(E_d)
      matmuls (512-free) on PE, one out DMA per tile.
"""

from contextlib import ExitStack

import numpy as np

import concourse.bass as bass
import concourse.tile as tile
from concourse import bacc, mybir
from concourse.masks import make_identity

B, N, C, ED, O, CHEB_K = 64, 2048, 64, 16, 64, 3
NCORES = 8
BL = B // NCORES  # 8 batches per core
BC = BL * C  # 512
NT = N // 128  # 16 node tiles
F32 = mybir.dt.float32
F32R = mybir.dt.float32r
BF16 = mybir.dt.bfloat16
AF = mybir.ActivationFunctionType
ALU = mybir.AluOpType


def build(debug=False):
    nc = bacc.Bacc(None)
    x = nc.declare_dram_parameter("x", [BL, N, C], F32, isOutput=False)
    emb = nc.declare_dram_parameter("emb", [N, ED], F32, isOutput=False)
    wp = nc.declare_dram_parameter("wp", [ED, CHEB_K, C, O], F32, isOutput=False)
    bp = nc.declare_dram_parameter("bp", [ED, O], F32, isOutput=False)
    out = nc.declare_dram_parameter("out", [BL, N, O], F32, isOutput=True)

    with tile.TileContext(nc) as tc, ExitStack() as ctx:
        const = ctx.enter_context(tc.tile_pool(name="const", bufs=1))
        zp = ctx.enter_context(tc.tile_pool(name="zp", bufs=1))

        # ---- constants -------------------------------------------------
        E_all = const.tile([128, NT, ED], F32, tag="E_all")  # [128, tile, d]
        for eq in range(4):
            nc.sync.dma_start(
                E_all[:, 4 * eq : 4 * (eq + 1)],
                emb.rearrange("(t p) d -> p t d", p=128)[:, 4 * eq : 4 * (eq + 1)],
            )
        ident_f = const.tile([128, 128], F32, tag="ident_f")
        make_identity(nc, ident_f[:])
        ident_b = const.tile([128, 128], BF16, tag="ident_b")
        make_identity(nc, ident_b[:])
        # E^T in f32r via PE transpose of the natural-layout tiles
        ETr = const.tile([ED, N], F32R, tag="ETr")
        with tc.tile_pool(name="etps", bufs=4, space="PSUM") as etps:
            for i in range(NT):
                pt_e = etps.tile([ED, 128], F32, tag="pt_e")
                nc.tensor.transpose(pt_e[:], E_all[:, i, :], ident_f[:])
                nc.vector.tensor_copy(ETr[:, 128 * i : 128 * (i + 1)], pt_e[:])
        bp_r = const.tile([ED, O], F32R, tag="bp_r")
        nc.gpsimd.dma_start(bp_r[:], bp[:, :])
        # weight pool, contraction-major: wpA rows = (k-1)*64+c for k=1,2
        wpA = const.tile([128, ED, O], BF16, tag="wpA")
        nc.gpsimd.dma_start(wpA[:], wp[:, 1:3].rearrange("d k c o -> (k c) d o"))
        # W0 - W2 (cheb T2's -I term), duplicated in both partition halves so
        # the x-part lhsT slice can sit at partition 0 or 64.
        wpB = const.tile([128, ED, O], BF16, tag="wpB")
        with tc.tile_pool(name="wtmp", bufs=1) as wtmp:
            wp0 = wtmp.tile([C, ED, O], F32, tag="wp0")
            nc.sync.dma_start(wp0[:], wp[:, 0].rearrange("d c o -> c d o"))
            wp2 = wtmp.tile([C, ED, O], F32, tag="wp2")
            nc.sync.dma_start(wp2[:], wp[:, 2].rearrange("d c o -> c d o"))
            nc.vector.tensor_sub(wpB[0:C], wp0[:], wp2[:])
            nc.vector.tensor_sub(wpB[C:128], wp0[:], wp2[:])

        s_parts = const.tile([128, NT, 2], F32, tag="s_parts")
        s_all = const.tile([128, NT], F32, tag="s_all")
        r_all = const.tile([128, NT], F32, tag="r_all")
        r2_all = const.tile([128, NT], F32, tag="r2_all")
        bias_all = const.tile([128, NT, O], F32, tag="bias_all")

        # ---- per-node bias: bias[n, o] = E[n] @ bias_pool --------------
        with tc.tile_pool(name="pbias", bufs=4, space="PSUM") as pbias:
            for i in range(NT):
                pt = pbias.tile([128, O], F32, tag="pb")
                nc.tensor.matmul(
                    pt[:], ETr[:, 128 * i : 128 * (i + 1)], bp_r[:], start=True, stop=True
                )
                nc.scalar.copy(bias_all[:, i], pt[:])

        # ---- persistent SBUF state ------------------------------------
        T_sb = [zp.tile([128, N], BF16, tag=f"T{j}", name=f"T{j}") for j in range(NT)]
        # z1/z2 interleaved per batch: one [128,128] transpose of ZZ[:, b]
        # yields the stacked GEMM lhsT.
        ZZ = [zp.tile([128, BL, 2, C], BF16, tag=f"ZZ{j}", name=f"ZZ{j}") for j in range(NT)]
        # pre-transposed x: XT[j][:, h] = [x_{2h}; x_{2h+1}]^T per batch pair
        XT = [zp.tile([128, BL // 2, 128], BF16, tag=f"XT{j}", name=f"XT{j}") for j in range(NT)]
        # stacked transposes, filled per tile at the top of S4
        stA = [
            zp.tile([128, BL, 128], BF16, tag=f"stA{i}", name=f"stA{i}")
            for i in range(NT)
        ]

        # Xb/Z1b live only through S2; their pool closes to make room for
        # the S4 y-ring.
        xzp_cm = tc.tile_pool(name="xzp", bufs=1)
        xzp = xzp_cm.__enter__()
        Xb = [xzp.tile([128, BL, C], BF16, tag=f"Xb{j}", name=f"Xb{j}") for j in range(NT)]
        Z1b = [xzp.tile([128, BL, C], BF16, tag=f"Z1b{j}", name=f"Z1b{j}") for j in range(NT)]

        # ---- S1: x load/cast + adjacency + XT transposes ---------------
        # z1-q0 moved to S2 so the freed PSUM banks deepen the adjacency
        # pipeline (bps bufs=4) and host the XT transpose staging -- PE
        # stays fed while relu/exp drain.
        with (
            tc.tile_pool(name="xst", bufs=4) as xst,
            tc.tile_pool(name="bps", bufs=2, space="PSUM") as bps,
            tc.tile_pool(name="xtps", bufs=4, space="PSUM") as xtps,
        ):
            for j in range(NT):
                xs = xst.tile([128, BL, C], F32, tag="xs")
                nc.sync.dma_start(
                    xs[:], x[:, 128 * j : 128 * (j + 1), :].rearrange("b p c -> p b c")
                )
                nc.gpsimd.tensor_copy(Xb[j][:], xs[:])
                # adjacency row tile: relu in place in PSUM (DVE), then ACT
                # exp straight to bf16 SBUF with free rowsum accum
                for h in range(2):
                    pe = bps.tile([128, 2, 512], F32, tag="pe")
                    for qq in range(2):
                        q = 2 * h + qq
                        nc.tensor.matmul(
                            pe[:, qq],
                            ETr[:, 128 * j : 128 * (j + 1)],
                            ETr[:, 512 * q : 512 * (q + 1)],
                            start=True,
                            stop=True,
                        )
                    # exp(relu(l)) = max(exp(l), 1): exp straight from PSUM
                    # (frees the bank early), then an in-place SBUF clamp on
                    # DVE whose accum_out yields the correctly clamped rowsum
                    tsl = T_sb[j][:, 1024 * h : 1024 * (h + 1)]
                    nc.scalar.activation(tsl, pe[:], AF.Exp)
                    nc.vector.scalar_tensor_tensor(
                        tsl,
                        tsl,
                        1.0,
                        tsl,
                        ALU.max,
                        ALU.max,
                        accum_out=s_parts[:, j, h : h + 1],
                    )
                # x-transposes for the tile loaded two iterations ago
                if j >= 2:
                    tj = j - 2
                    xfj = Xb[tj][:].rearrange("p b c -> p (b c)")
                    for th in range(4):
                        ptx = xtps.tile([128, 128], BF16, tag="ptx")
                        nc.tensor.transpose(
                            ptx[:], xfj[:, 128 * th : 128 * (th + 1)], ident_b[:]
                        )
                        if th % 2 == 0:
                            nc.scalar.copy(XT[tj][:, th], ptx[:])
                        else:
                            nc.vector.tensor_copy(XT[tj][:, th], ptx[:])
            for tj in (NT - 2, NT - 1):
                xfj = Xb[tj][:].rearrange("p b c -> p (b c)")
                for th in range(4):
                    ptx = xtps.tile([128, 128], BF16, tag="ptx")
                    nc.tensor.transpose(
                        ptx[:], xfj[:, 128 * th : 128 * (th + 1)], ident_b[:]
                    )
                    if th % 2 == 0:
                        nc.scalar.copy(XT[tj][:, th], ptx[:])
                    else:
                        nc.vector.tensor_copy(XT[tj][:, th], ptx[:])
            nc.vector.tensor_reduce(
                s_all[:], s_parts[:], axis=mybir.AxisListType.X, op=ALU.add
            )
            nc.vector.reciprocal(r_all[:], s_all[:])
            nc.vector.tensor_scalar_mul(r2_all[:], r_all[:], 2.0)

        def evict1(i, psk):
            # bf16 copies for the z2-pass rhs and the stacked GEMM
            # transposes, both scaled by r. ACT is idle here.
            nc.scalar.activation(
                Z1b[i][:], psk[:], AF.Copy, scale=r_all[:, i : i + 1]
            )
            nc.scalar.activation(
                ZZ[i][:, :, 0, :], psk[:], AF.Copy, scale=r_all[:, i : i + 1]
            )

        # ---- S2: z1 sweeps then z2 sweeps, one shared PSUM pool --------
        # Same tile tags across all eight q-groups: no pool-close barrier
        # between z1 and z2, so the PE stream never drains (p-state stays
        # at full clock).
        with tc.tile_pool(name="zps", bufs=2, space="PSUM") as zps:
            for q in range(4):
                ps = [
                    zps.tile([128, BL, C], F32, tag=f"ps{k}", name=f"ps{k}")
                    for k in range(4)
                ]
                for j in range(NT):
                    for k in range(4):
                        nc.tensor.matmul(
                            ps[k][:],
                            T_sb[j][:, 512 * q + 128 * k : 512 * q + 128 * (k + 1)],
                            Xb[j][:],
                            start=(j == 0),
                            stop=(j == NT - 1),
                        )
                for k in range(4):
                    evict1(4 * q + k, ps[k])
            for q in range(4):
                ps = [
                    zps.tile([128, BL, C], F32, tag=f"ps{k}", name=f"ps{k}")
                    for k in range(4)
                ]
                for j in range(NT):
                    for k in range(4):
                        nc.tensor.matmul(
                            ps[k][:],
                            T_sb[j][:, 512 * q + 128 * k : 512 * q + 128 * (k + 1)],
                            Z1b[j][:],
                            start=(j == 0),
                            stop=(j == NT - 1),
                        )
                for k in range(4):
                    i = 4 * q + k
                    nc.scalar.activation(
                        ZZ[i][:, :, 1, :], ps[k][:], AF.Copy, scale=r2_all[:, i : i + 1]
                    )
        xzp_cm.__exit__(None, None, None)

        # ---- S4: GEMM units + PE diag d-contraction --------------------
        with (
            tc.tile_pool(name="yp", bufs=3, space="PSUM") as ypp,
            tc.tile_pool(name="trp", bufs=1, space="PSUM") as trp,
            tc.tile_pool(name="pdp", bufs=1, space="PSUM") as pdp,
            tc.tile_pool(name="ysb", bufs=2) as ysb,
            tc.tile_pool(name="dgp", bufs=2) as dgp,
            tc.tile_pool(name="bbp", bufs=1) as bbp,
            tc.tile_pool(name="obp", bufs=2) as obp,
        ):
            for i in range(NT):
                # per-tile constants, built on ACT/DVE/Pool (all have slack)
                dgs = []
                for d in range(16):
                    dg = dgp.tile([128, 128], BF16, tag=f"dg{d}", name=f"dg{d}")
                    nc.vector.tensor_scalar_mul(
                        dg[:], ident_b[:], E_all[:, i, d : d + 1]
                    )
                    dgs.append(dg)
                # stacked [z1_b; z2_b]^T: PE transposes into a 1-bank PSUM
                # staging tile, evicted to SBUF on ACT/DVE
                ptr = trp.tile([128, BL, 128], BF16, tag="ptr", name="ptr")
                for b in range(BL):
                    nc.tensor.transpose(
                        ptr[:, b], ZZ[i][:, b].rearrange("p s c -> p (s c)"), ident_b[:]
                    )
                    if b % 2 == 0:
                        nc.scalar.copy(stA[i][:, b], ptr[:, b])
                    else:
                        nc.vector.tensor_copy(stA[i][:, b], ptr[:, b])
                # GEMM units: (batch-pair, d-half) -> 2-bank PSUM, bulk evict
                ysb_t = ysb.tile([128, 4, 2, 2 * 512], BF16, tag="ysb", name="ysb")
                for g2 in range(4):
                    for ch in range(2):
                        dsl = slice(8 * ch, 8 * (ch + 1))
                        py = ypp.tile([128, 2, 512], F32, tag="py", name="py")
                        for p in range(2):
                            nc.tensor.matmul(
                                py[:, p],
                                stA[i][:, 2 * g2 + p],
                                wpA[:, dsl],
                                start=True,
                                stop=False,
                            )
                            off = C * (p % 2)
                            nc.tensor.matmul(
                                py[:, p],
                                XT[i][64 * (p % 2) : 64 * (p % 2) + 64, g2, :],
                                wpB[off : off + C, dsl],
                                start=False,
                                stop=True,
                            )
                        dst = ysb_t[:, g2, ch].rearrange("p (b f) -> p b f", b=2)
                        if (g2 + ch) % 2 == 0:
                            nc.scalar.copy(dst, py[:])
                        else:
                            nc.vector.tensor_copy(dst, py[:])
                # d-contraction + bias: 17 diag matmuls, 512-free each
                yv = ysb_t[:].rearrange("p g2 ch (b f) -> p g2 ch b f", b=2)
                pd = pdp.tile([128, BL, O], F32, tag="pd", name="pd")
                pdv = pd[:].rearrange("p b o -> p (b o)").rearrange(
                    "p (g2 b o) -> p g2 b o", g2=4, b=2
                )
                # two independent accumulation groups (g2-pairs) so the
                # first half's d-contraction overlaps the second half's
                # GEMM units; each half evicts + DMAs as soon as it stops
                outF = obp.tile([128, BL, O], F32, tag="outF", name="outF")
                for half in range(2):
                    for ch in range(2):
                        for dl in range(8):
                            nc.tensor.matmul(
                                pdv[:, 2 * half : 2 * half + 2],
                                dgs[8 * ch + dl][:],
                                yv[:, 2 * half : 2 * half + 2, ch, :, O * dl : O * (dl + 1)],
                                start=(ch == 0 and dl == 0),
                                stop=(ch == 1 and dl == 7),
                            )
                    hb = slice(4 * half, 4 * half + 4)
                    nc.vector.scalar_tensor_tensor(
                        outF[:, hb, :],
                        pd[:, hb, :],
                        1.0,
                        bias_all[:, i : i + 1, :].broadcast_to([128, 4, O]),
                        ALU.mult,
                        ALU.add,
                    )
                    nc.sync.dma_start(
                        out[hb, 128 * i : 128 * (i + 1), :].rearrange("b p o -> p b o"),
                        outF[:, hb, :],
                    )

    nc.finalize()
    return nc


_NC_CACHE = {}


def kernel(x, node_embeddings, weights_pool, bias_pool):
    from concourse.bass_utils import run_bass_kernel_spmd

    if "nc" not in _NC_CACHE:
        _NC_CACHE["nc"] = build()
    nc = _NC_CACHE["nc"]

    x = np.asarray(x, dtype=np.float32)
    emb = np.asarray(node_embeddings, dtype=np.float32)
    wp = np.asarray(weights_pool, dtype=np.float32)
    bp = np.asarray(bias_pool, dtype=np.float32)

    in_maps = [
        {"x": x[ci * BL : (ci + 1) * BL], "emb": emb, "wp": wp, "bp": bp}
        for ci in range(NCORES)
    ]
    res = run_bass_kernel_spmd(nc, in_maps, list(range(NCORES)))
    return np.concatenate([res.results[ci]["out"] for ci in range(NCORES)], axis=0)

